# revision 1
# baseline (speedup 1.0000x reference)
"""Trainium2 Bass kernel for nn_LowRankSig_HigherOrder.

Math (per example, T=2048, U=64, F=64 incl. time channel):
  Xa  = concat(time, X)                      [T, 64]
  dXa = diff(Xa) (zero row at t=0)
  M_k = dXa @ K_k                            (K_k = kernel[:, k, :], k=0..9)
  E_k = ecum_t(M_k) = Ya @ K_k   with  Ya[t] = Xa[t-1] - Xa[0]   (linearity)
  out = sum_t M_0                = (Xa[T-1]-Xa[0]) @ K_0
      + sum_t [ M2*E1 + 1/2 M2*M1 ]
      + sum_t [ M5*EA2 + 1/2 M5*R1a + 1/3 M5*R1b ],  R1a=M4*E3, R1b=1/2 M4*M3,
            EA2 = ecum(R1a+R1b)
      + sum_t [ M9*EB3 + 1/2 M9*Sa + 1/3 M9*Sb + 1/4 M9*Sc ],
            Ra=M7*E6, Rb=1/2 M7*M6, EB2=ecum(Ra+Rb),
            Sa=M8*EB2, Sb=1/2 M8*Ra, Sc=1/3 M8*Rb, EB3=ecum(Sa+Sb+Sc)

Sharding: pure data parallel, 4 examples per core, packed 2-per-128-partitions.
Device layout: [2*64 = 128 partitions (example, feature/unit), T on free dim].
Projections run on the PE with block-diagonal [128,128] kernel slices; the three
product-chain cumsums use the DVE tensor_tensor_scan (fp32 state); per-term
time-sums are fused into the products via scalar_tensor_tensor accum_out.
"""

import numpy as np

import concourse.bass as bass
import concourse.mybir as mybir
import concourse.tile as tile
from concourse.bass_utils import run_bass_kernel_spmd
from bass_rust import ScopedClock


def _patched_drain_and_barrier(self, tick_clock, wait_clock):
    """Split the final drain's sem waits across multiple drain instructions.

    The walrus build in this container rejects instructions carrying more
    than a couple of sync waits ("Too many sync wait commands"); Tile's
    default exit path puts one wait per outstanding proc on a single Drain.
    Sequential same-engine drains each carrying one wait are semantically
    identical.
    """
    drain_inst = self.nc.sync.drain()
    wait_clock.add_sem_waits(drain_inst.ins, ScopedClock({None: tick_clock.global_clock}))
    si = drain_inst.ins.sync_info
    if si is not None and si.on_wait and len(si.on_wait) > 1:
        waits = list(si.on_wait)
        ups = list(si.on_update or [])
        drain_inst.ins.sync_info = mybir.SyncInfo(on_wait=waits[:1], on_update=ups)
        for w in waits[1:]:
            d2 = self.nc.sync.drain()
            d2.ins.sync_info = mybir.SyncInfo(on_wait=[w], on_update=[])

    self.nc.all_engine_barrier()
    popped = self.nc._tile_sem_poison_stack.pop()
    assert popped is self._sem_poison
    self.nc.clear_and_free_semaphores(list(self.sems.allocated().values()))
    self.nc.all_engine_barrier()


tile.TileContext._drain_and_barrier = _patched_drain_and_barrier


def _sanitize_waits(nc, limit=1):
    """Move excess sem waits onto same-engine NOPs inserted just before.

    This walrus build rejects instructions with more than ~1-2 sync waits;
    a NOP that blocks the engine on the extra sems first is equivalent.
    """
    import bass_rust

    counter = [0]
    for f in nc.m.functions:
        for blk in f.blocks:
            il = blk.instructions
            i = 0
            while i < len(il):
                inst = il[i]
                si = inst.sync_info
                waits = list(si.on_wait) if (si is not None and si.on_wait) else []
                if len(waits) > limit:
                    keep, extra = waits[:limit], waits[limit:]
                    inst.sync_info = mybir.SyncInfo(
                        on_wait=keep, on_update=list(si.on_update or [])
                    )
                    for j in range(0, len(extra), limit):
                        counter[0] += 1
                        nop = bass_rust.InstNoOp(
                            name=f"waitnop-{counter[0]}", ins=[], outs=[]
                        )
                        nop.engine = inst.engine
                        nop.sync_info = mybir.SyncInfo(
                            on_wait=extra[j : j + limit], on_update=[]
                        )
                        il.insert(i, nop)
                        i += 1
                i += 1
    return counter[0]

B, T, FX = 32, 2048, 63
U = 64
LT = 10
NCORES = 8
BL = B // NCORES  # 4 examples per core
NPAIR = BL // 2   # 2 partition-packed pairs per core

FP = mybir.dt.float32
BF = mybir.dt.float16
# Matmul passes: 3 = hi@Khi + lo@Khi + hi@Klo (split fp32, ~1e-4),
# 2 = hi@Khi + lo@Khi, 1 = plain bf16.
MM_PASSES = 3
# Elementwise dtype for the product/scan phase (FP exact; BF = 2x DVE rate).
ELEM_DT = FP
AluOp = mybir.AluOpType


def _prep_pair(nc, pool, xp_d, p):
    """Load + prep one pair's inputs (emitted for both pairs up front so
    pair 1's projections can start as soon as the PE frees up)."""
    fp = FP
    TT = T

    xa = pool.tile([128, TT], fp, tag="xa", name="xa")
    nc.sync.dma_start(xa[:], xp_d[p])

    # Ya first (feeds E1, the first product's input), then dXa
    ya = pool.tile([128, TT], fp, tag="ya", name="ya")
    nc.gpsimd.memset(ya[:, 0:1], 0.0)
    nc.vector.tensor_scalar(
        out=ya[:, 1:TT],
        in0=xa[:, 0 : TT - 1],
        scalar1=xa[:, 0:1],
        scalar2=None,
        op0=AluOp.subtract,
    )
    dx = pool.tile([128, TT], fp, tag="dx", name="dx")
    nc.gpsimd.memset(dx[:, 0:1], 0.0)
    nc.vector.tensor_tensor(
        out=dx[:, 1:TT], in0=xa[:, 1:TT], in1=xa[:, 0 : TT - 1], op=AluOp.subtract
    )
    d0 = pool.tile([128, 1], fp, tag="d0", name="d0")
    nc.gpsimd.tensor_scalar(
        out=d0[:, 0:1],
        in0=xa[:, TT - 1 : TT],
        scalar1=xa[:, 0:1],
        scalar2=None,
        op0=AluOp.subtract,
    )


    # bf16 split of the matmul moving operands: x = hi + lo (+ ~1e-7 f32 tail)
    def split(src_t, nm):
        hi = pool.tile([128, TT], BF, tag=nm + "h", name=nm + "h", bufs=2)
        nc.scalar.copy(out=hi[:], in_=src_t[:])
        lo = None
        if MM_PASSES >= 2:
            lo = pool.tile([128, TT], BF, tag=nm + "l", name=nm + "l", bufs=2)
            nc.gpsimd.tensor_tensor(
                out=lo[:], in0=src_t[:], in1=hi[:], op=AluOp.subtract
            )
        return hi, lo

    yah, yal = split(ya, "ya")
    dxh, dxl = split(dx, "dx")
    d0h = pool.tile([128, 1], BF, tag="d0h", name="d0h")
    nc.vector.tensor_copy(out=d0h[:], in_=d0[:])
    d0l = None
    if MM_PASSES >= 2:
        d0l = pool.tile([128, 1], BF, tag="d0l", name="d0l")
        nc.gpsimd.tensor_tensor(out=d0l[:], in0=d0[:], in1=d0h[:], op=AluOp.subtract)
    return dxh, dxl, yah, yal, d0h, d0l


def _compute_pair(nc, pool, psum, s0psum, kbt, prep, out_d, p):
    fp = FP
    TT = T
    dxh, dxl, yah, yal, d0h, d0l = prep
    kbth, kbtl = kbt
    def mm_group(ps_ap, k, rhsh, rhsl, n):
        kh = kbth[:, k * 128 : (k + 1) * 128]
        nc.tensor.matmul(ps_ap, kh, rhsh, start=True, stop=(MM_PASSES == 1))
        if MM_PASSES >= 2:
            nc.tensor.matmul(ps_ap, kh, rhsl, start=False, stop=(MM_PASSES == 2))
        if MM_PASSES >= 3:
            kl = kbtl[:, k * 128 : (k + 1) * 128]
            nc.tensor.matmul(ps_ap, kl, rhsh, start=False, stop=True)

    def project(name, which, k, tag, bufs=None):
        rh = dxh if which == "dx" else yah
        rl = dxl if which == "dx" else yal
        dst = pool.tile([128, TT], ELEM_DT, tag=tag, name=name, bufs=bufs)
        for h in range(2):  # two [128, 1024] psum tiles per slice
            ps = psum.tile([128, 1024], fp, tag="mm", name="mm")
            for j in range(2):  # one PSUM bank (512 fp32) per matmul group
                lo = h * 1024 + j * 512
                mm_group(
                    ps[:, j * 512 : (j + 1) * 512],
                    k,
                    rh[:, lo : lo + 512],
                    rl[:, lo : lo + 512] if rl is not None else None,
                    512,
                )
            nc.scalar.copy(out=dst[:, h * 1024 : (h + 1) * 1024], in_=ps[:])
        return dst

    # order roughly by consumption order
    E1 = project("E1", "ya", 1, "mD", bufs=2)
    M2 = project("M2", "dx", 2, "mB")
    M1 = project("M1", "dx", 1, "mA")
    E3 = project("E3", "ya", 3, "mE")
    M4 = project("M4", "dx", 4, "mF")
    M3 = project("M3", "dx", 3, "mC", bufs=2)
    M5 = project("M5", "dx", 5, "mG", bufs=2)
    E6 = project("E6", "ya", 6, "mE")
    M7 = project("M7", "dx", 7, "mB")
    M6 = project("M6", "dx", 6, "mA")
    M8 = project("M8", "dx", 8, "mC", bufs=2)
    M9 = project("M9", "dx", 9, "mD", bufs=2)

    # sum_t M0 via d0 @ K0 (tiny matmul, own psum bank)
    s0 = s0psum.tile([128, 1], fp, tag="s0", name="s0")
    mm_group(s0[:], 0, d0h[:], d0l[:] if d0l is not None else None, 1)

    acc = pool.tile([128, 12], fp, tag="acc", name="acc")

    def stt(out, in0, s, in1, op1=AluOp.mult, acc_col=None, engine=None):
        eng = engine if engine is not None else nc.vector
        eng.scalar_tensor_tensor(
            out=out[:],
            in0=in0[:],
            scalar=float(s),
            in1=in1[:],
            op0=AluOp.mult,
            op1=op1,
            accum_out=acc[:, acc_col : acc_col + 1] if acc_col is not None else None,
        )

    def scratch(i):
        return pool.tile([128, TT], BF, tag="scr", name=f"scr{i}", bufs=1)

    def scan_exc(name, d0t, d1t, tag, bufs=None):
        out = pool.tile([128, TT], ELEM_DT, tag=tag, name=name, bufs=bufs)
        nc.gpsimd.memset(out[:, 0:1], 0.0)
        nc.vector.tensor_tensor_scan(
            out=out[:, 1:TT],
            data0=d0t[:, 0 : TT - 1],
            data1=d1t[:, 0 : TT - 1],
            initial=0.0,
            op0=AluOp.add,
            op1=AluOp.add,
        )
        return out

    # ---- level m=1 ----
    stt(scratch(0), E1, 1.0, M2, acc_col=0)
    stt(scratch(1), M1, 0.5, M2, acc_col=1)

    # ---- level m=2 ----
    R1a = pool.tile([128, TT], ELEM_DT, tag="rA", name="R1a")
    stt(R1a, E3, 1.0, M4)
    R1b = pool.tile([128, TT], ELEM_DT, tag="rB", name="R1b", bufs=2)
    stt(R1b, M3, 0.5, M4)
    EA2 = scan_exc("EA2", R1a, R1b, "eA")
    stt(scratch(0), EA2, 1.0, M5, acc_col=2)
    stt(scratch(1), R1a, 0.5, M5, acc_col=3)
    stt(scratch(2), R1b, 1.0 / 3.0, M5, acc_col=4)

    # ---- level m=3 ----
    Ra = pool.tile([128, TT], ELEM_DT, tag="Ra", name="Ra")
    stt(Ra, E6, 1.0, M7)
    Rb = pool.tile([128, TT], ELEM_DT, tag="Rb", name="Rb")
    stt(Rb, M6, 0.5, M7)
    EB2 = scan_exc("EB2", Ra, Rb, "mF")
    Sa = pool.tile([128, TT], ELEM_DT, tag="rA", name="Sa")
    stt(Sa, EB2, 1.0, M8)
    Sb = pool.tile([128, TT], ELEM_DT, tag="rB", name="Sb", bufs=2)
    stt(Sb, Ra, 0.5, M8)
    Sc = pool.tile([128, TT], ELEM_DT, tag="eA", name="Sc")
    stt(Sc, Rb, 1.0 / 3.0, M8)
    Tab = pool.tile([128, TT], ELEM_DT, tag="Tab", name="Tab")
    nc.vector.tensor_tensor(out=Tab[:], in0=Sa[:], in1=Sb[:], op=AluOp.add)
    EB3 = scan_exc("EB3", Tab, Sc, "mG", bufs=2)
    stt(scratch(0), EB3, 1.0, M9, acc_col=5)
    stt(scratch(1), Sa, 0.5, M9, acc_col=6)
    stt(scratch(2), Sb, 1.0 / 3.0, M9, acc_col=7)
    stt(scratch(3), Sc, 0.25, M9, acc_col=8)

    # ---- final reduce: out = s0 + sum(acc[:, 0:9]) ----
    red = pool.tile([128, 1], fp, tag="red", name="red")
    nc.vector.tensor_reduce(
        out=red[:], in_=acc[:, 0:9], axis=mybir.AxisListType.X, op=AluOp.add
    )
    outt = pool.tile([128, 1], fp, tag="outt", name="outt")
    nc.vector.tensor_tensor(out=outt[:], in0=red[:], in1=s0[:], op=AluOp.add)
    nc.sync.dma_start(out_d[p], outt[:])


def build_nc(sanitize=True):
    nc = bass.Bass("TRN2", target_bir_lowering=False, debug=False)
    xp_d = nc.dram_tensor("xp", [NPAIR, 128, T], FP, kind="ExternalInput")
    kbh_d = nc.dram_tensor("kbh", [LT, 128, 128], BF, kind="ExternalInput")
    kbl_d = nc.dram_tensor("kbl", [LT, 128, 128], BF, kind="ExternalInput")
    out_d = nc.dram_tensor("out", [NPAIR, 128, 1], FP, kind="ExternalOutput")

    with tile.TileContext(nc) as tc:
        with (
            tc.tile_pool(name="pool", bufs=1) as pool,
            tc.tile_pool(name="psum", bufs=3, space="PSUM") as psum,
            tc.tile_pool(name="s0psum", bufs=1, space="PSUM") as s0psum,
            tc.tile_pool(name="kpool", bufs=1) as kpool,
        ):
            kbth = kpool.tile([128, LT * 128], BF, tag="kbth", name="kbth")
            nc.sync.dma_start(
                kbth[:].rearrange("p (k m) -> p k m", k=LT),
                kbh_d.ap().rearrange("k f m -> f k m"),
            )
            kbtl = kpool.tile([128, LT * 128], BF, tag="kbtl", name="kbtl")
            nc.sync.dma_start(
                kbtl[:].rearrange("p (k m) -> p k m", k=LT),
                kbl_d.ap().rearrange("k f m -> f k m"),
            )
            preps = [_prep_pair(nc, pool, xp_d, p) for p in range(NPAIR)]
            for p in range(NPAIR):
                _compute_pair(
                    nc, pool, psum, s0psum, (kbth, kbtl), preps[p], out_d, p
                )
    if sanitize:
        n = _sanitize_waits(nc)
        print(f"[kernel] split {n} excess sem waits onto NOPs")
    return nc


_CACHE = {}


def _get_nc():
    if "nc" not in _CACHE:
        _CACHE["nc"] = build_nc()
    return _CACHE["nc"]


def _marshal(X, kernel):
    """Host-side input marshaling: time channel, transpose-pack, shard."""
    Xf = np.ascontiguousarray(X, dtype=np.float32)
    tch = (np.arange(T, dtype=np.float32) * (2.0 / (T - 1.0)) - 1.0)
    Xa = np.empty((B, T, U), dtype=np.float32)
    Xa[:, :, 0] = tch[None, :]
    Xa[:, :, 1:] = Xf
    # [core, pair, ex, t, f] -> [core, pair, ex, f, t] -> [core, pair, 128, T]
    xp = np.ascontiguousarray(
        Xa.reshape(NCORES, NPAIR, 2, T, U).transpose(0, 1, 2, 4, 3)
    ).reshape(NCORES, NPAIR, 128, T)

    kf = np.asarray(kernel, dtype=np.float32)  # [64, 10, 64]
    kb = np.zeros((LT, 128, 128), dtype=np.float32)
    kb[:, :U, :U] = kf.transpose(1, 0, 2)
    kb[:, U:, U:] = kf.transpose(1, 0, 2)
    kbh = kb.astype(np.float16)
    kbl = (kb - kbh.astype(np.float32)).astype(np.float16)
    return xp, kbh, kbl


def run(X, kernel, trace=False):
    nc = _get_nc()
    xp, kbh, kbl = _marshal(X, kernel)
    in_maps = [{"xp": xp[c], "kbh": kbh, "kbl": kbl} for c in range(NCORES)]
    res = run_bass_kernel_spmd(nc, in_maps, list(range(NCORES)), trace=trace)
    out = np.stack([r["out"] for r in res.results])  # [8, NPAIR, 128, 1]
    out = out.reshape(NCORES, NPAIR, 2, U).reshape(B, U)
    return out, res


def kernel(X, kernel):
    out, _ = run(X, kernel)
    return out



# revision 6
# speedup vs baseline: 1.0610x; 1.0610x over previous
"""Trainium2 Bass kernel for nn_LowRankSig_HigherOrder (v3).

Math (per example, T=2048, U=64, F=64 incl. time channel), all constants
folded into host-precomputed fp16 moving streams so the device needs only
plain tensor_tensor products (DVE 2x fp16 mode), one scan, and fused
tensor_tensor_reduce accumulations:

  dXa[t] = Xa[t]-Xa[t-1] (0 at t=0)         Ya[t] = Xa[t-1]-Xa[0] (0 at t=0)
  Za = Ya + dXa/2      (Za@Kk = E_k + M_k/2)
  Wa = Ya/2 + dXa/6    (Wa@Kk = E_k/2 + M_k/6)
  Va = Ya/6 + dXa/24   (Va@Kk = E_k/6 + M_k/24)
  Ha[t] = Xa[T-1]-Xa[t]  (Ha@Kk = reverse-exclusive-cumsum of M_k)

  out = d0@K0                                   (s0, 1-col matmul)
      + sum_t M2*(Za@K1)                        (level 1)
      + sum_t (M4*Za@K3)*(Ha@K5) + M5*(M4*Wa@K3)        (level 2, by parts)
      + sum_t T3*(Ha@K9) + M9*T3''              (level 3, by parts on EB3)
        where P3 = M7*Za@K6, EB2 = ecum(P3), Q3 = M7*Wa@K6, Q3' = M7*Va@K6,
              T3 = M8*(EB2+Q3), T3'' = M8*(EB2/2 + Q3')

Sharding: pure data parallel, 4 examples/core, 2 examples packed per 128
partitions (block-diagonal kernel). Single-pass fp16 matmuls; multi-use
projections drained PSUM->SBUF fp16 by the ACT engine; single-use
projections consumed directly from PSUM by chunked tensor_tensor_reduce.
"""

import numpy as np

import concourse.bass as bass
import concourse.mybir as mybir
import concourse.tile as tile
from concourse.bass_utils import run_bass_kernel_spmd
from bass_rust import ScopedClock


def _patched_drain_and_barrier(self, tick_clock, wait_clock):
    """Split the final drain's sem waits across multiple drain instructions
    (walrus build rejects >1-2 sync waits per instruction)."""
    drain_inst = self.nc.sync.drain()
    wait_clock.add_sem_waits(drain_inst.ins, ScopedClock({None: tick_clock.global_clock}))
    si = drain_inst.ins.sync_info
    if si is not None and si.on_wait and len(si.on_wait) > 1:
        waits = list(si.on_wait)
        ups = list(si.on_update or [])
        drain_inst.ins.sync_info = mybir.SyncInfo(on_wait=waits[:1], on_update=ups)
        for w in waits[1:]:
            d2 = self.nc.sync.drain()
            d2.ins.sync_info = mybir.SyncInfo(on_wait=[w], on_update=[])

    self.nc.all_engine_barrier()
    popped = self.nc._tile_sem_poison_stack.pop()
    assert popped is self._sem_poison
    self.nc.clear_and_free_semaphores(list(self.sems.allocated().values()))
    self.nc.all_engine_barrier()


tile.TileContext._drain_and_barrier = _patched_drain_and_barrier


def _sanitize_waits(nc, limit=1):
    """Move excess sem waits onto same-engine NOPs inserted just before."""
    import bass_rust

    counter = [0]
    for f in nc.m.functions:
        for blk in f.blocks:
            il = blk.instructions
            i = 0
            while i < len(il):
                inst = il[i]
                si = inst.sync_info
                waits = list(si.on_wait) if (si is not None and si.on_wait) else []
                if len(waits) > limit:
                    keep, extra = waits[:limit], waits[limit:]
                    inst.sync_info = mybir.SyncInfo(
                        on_wait=keep, on_update=list(si.on_update or [])
                    )
                    for j in range(0, len(extra), limit):
                        counter[0] += 1
                        nop = bass_rust.InstNoOp(
                            name=f"waitnop-{counter[0]}", ins=[], outs=[]
                        )
                        nop.engine = inst.engine
                        nop.sync_info = mybir.SyncInfo(
                            on_wait=extra[j : j + limit], on_update=[]
                        )
                        il.insert(i, nop)
                        i += 1
                i += 1
    return counter[0]


B, T, FX = 32, 2048, 63
U = 64
LT = 10
NCORES = 8
BL = B // NCORES  # 4 examples per core
NPAIR = BL // 2   # 2 partition-packed pairs per core

FP = mybir.dt.float32
F16 = mybir.dt.float16
AluOp = mybir.AluOpType

STREAMS = ["dxa", "za", "wa", "va", "ha"]


def _mm(nc, ps_ap, kb, k, rhs_ap):
    nc.tensor.matmul(ps_ap, kb[:, k * 128 : (k + 1) * 128], rhs_ap, start=True, stop=True)


class Pair:
    """Per-pair tile state."""

    def __init__(self, nc, pool, kb, dr, p):
        self.nc, self.pool, self.kb, self.p = nc, pool, kb, p
        # stream tiles, DMA'd in halves
        self.st = {}
        for s in STREAMS:
            t = pool.tile([128, T], F16, tag=f"{s}{p}", name=f"{s}{p}")
            for h in range(2):
                sl = slice(h * 1024, (h + 1) * 1024)
                nc.sync.dma_start(t[:, sl], dr[s][p][:, sl])
            self.st[s] = t
        self.d0 = pool.tile([128, 1], F16, tag=f"d0_{p}", name=f"d0_{p}")
        nc.sync.dma_start(self.d0[:], dr["d0"][p])
        self.acc = pool.tile([128, 24], FP, tag=f"acc{p}", name=f"acc{p}")
        self.sb = {}

    def tile(self, nm, dtype=F16, cols=T, bufs=None, tag=None):
        t = self.pool.tile([128, cols], dtype, tag=f"{tag or nm}{self.p}",
                           name=f"{nm}{self.p}", bufs=bufs)
        self.sb[nm] = t
        return t


def _proj_copied(nc, psA, pair, nm, stream, k):
    """Project stream@Kk, drain PSUM -> SBUF fp16 via ACT. Returns SBUF tile."""
    dst = pair.tile(nm)
    src = pair.st[stream]
    for h in range(2):
        ps = psA.tile([128, 1024], FP, tag="psA", name=f"psA_{nm}{pair.p}")
        for j in range(2):
            lo = h * 1024 + j * 512
            _mm(nc, ps[:, j * 512 : (j + 1) * 512], pair.kb, k, src[:, lo : lo + 512])
        nc.scalar.copy(out=dst[:, h * 1024 : (h + 1) * 1024], in_=ps[:])
    return dst


def _proj_ttr(nc, psB, scrap, pair, stream, k, other_sb, cols):
    """Project stream@Kk into PSUM chunks and immediately reduce
    sum_t(proj * other_sb) into acc[:, cols] via tensor_tensor_reduce."""
    src = pair.st[stream]
    for j in range(4):
        lo = j * 512
        ps = psB.tile([128, 512], FP, tag="psB", name=f"psB_{stream}{k}_{pair.p}")
        _mm(nc, ps[:], pair.kb, k, src[:, lo : lo + 512])
        sc = scrap.tile([128, 512], F16, tag="scrap", name=f"sc_{stream}{k}_{j}_{pair.p}")
        nc.vector.scalar_tensor_tensor(
            out=sc[:], in0=ps[:], scalar=1.0, in1=other_sb[:, lo : lo + 512],
            op0=AluOp.mult, op1=AluOp.mult,
            accum_out=pair.acc[:, cols + j : cols + j + 1],
        )


def build_nc(sanitize=True):
    nc = bass.Bass("TRN2", target_bir_lowering=False, debug=False)
    dr = {}
    for s in STREAMS:
        dr[s] = nc.dram_tensor(s, [NPAIR, 128, T], F16, kind="ExternalInput")
    dr["d0"] = nc.dram_tensor("d0", [NPAIR, 128, 1], F16, kind="ExternalInput")
    kb_d = nc.dram_tensor("kb", [LT, 128, 128], F16, kind="ExternalInput")
    out_d = nc.dram_tensor("out", [NPAIR, 128, 1], FP, kind="ExternalOutput")

    with tile.TileContext(nc) as tc:
        with (
            tc.tile_pool(name="pool", bufs=1) as pool,
            tc.tile_pool(name="scrap", bufs=4) as scrap,
            tc.tile_pool(name="psA", bufs=3, space="PSUM") as psA,
            tc.tile_pool(name="psB", bufs=2, space="PSUM") as psB,
        ):
            kb = pool.tile([128, LT * 128], F16, tag="kb", name="kb")
            nc.sync.dma_start(
                kb[:].rearrange("p (k m) -> p k m", k=LT),
                kb_d.ap().rearrange("k f m -> f k m"),
            )
            zeros = pool.tile([128, T], F16, tag="zeros", name="zeros")
            nc.gpsimd.memset(zeros[:], 0.0)

            pairs = [Pair(nc, pool, kb, dr, p) for p in range(NPAIR)]

            # ---- PASS 1 per pair ----
            for pr in pairs:
                p = pr.p
                # copied projections, phase 1
                M4 = _proj_copied(nc, psA, pr, "M4", "dxa", 4)
                ZK3 = _proj_copied(nc, psA, pr, "ZK3", "za", 3)
                WK3 = _proj_copied(nc, psA, pr, "WK3", "wa", 3)
                ZK1 = _proj_copied(nc, psA, pr, "ZK1", "za", 1)
                # level 1: acc1 = sum M2 * ZK1   (M2 direct from PSUM)
                _proj_ttr(nc, psB, scrap, pr, "dxa", 2, ZK1, 0)
                # level 2 products
                P2 = pr.tile("P2")
                nc.vector.tensor_tensor(out=P2[:], in0=M4[:], in1=ZK3[:], op=AluOp.mult)
                Q2 = pr.tile("Q2")
                nc.vector.tensor_tensor(out=Q2[:], in0=M4[:], in1=WK3[:], op=AluOp.mult)
                # acc2b = sum M5 * Q2 ; acc2a = sum HK5 * P2  (M5, HK5 direct)
                _proj_ttr(nc, psB, scrap, pr, "dxa", 5, Q2, 4)
                _proj_ttr(nc, psB, scrap, pr, "ha", 5, P2, 8)
                # copied projections, phase 2
                M7 = _proj_copied(nc, psA, pr, "M7", "dxa", 7)
                ZK6 = _proj_copied(nc, psA, pr, "ZK6", "za", 6)
                WK6 = _proj_copied(nc, psA, pr, "WK6", "wa", 6)
                VK6 = _proj_copied(nc, psA, pr, "VK6", "va", 6)
                M8 = _proj_copied(nc, psA, pr, "M8", "dxa", 8)
                # level 3 chain
                P3 = pr.tile("P3")
                nc.vector.tensor_tensor(out=P3[:], in0=M7[:], in1=ZK6[:], op=AluOp.mult)
                EB2 = pr.tile("EB2")
                nc.gpsimd.memset(EB2[:, 0:1], 0.0)
                nc.vector.tensor_tensor_scan(
                    out=EB2[:, 1:T], data0=P3[:, 0 : T - 1], data1=zeros[:, 0 : T - 1],
                    initial=0.0, op0=AluOp.add, op1=AluOp.add,
                )
                Q3 = pr.tile("Q3")
                nc.vector.tensor_tensor(out=Q3[:], in0=M7[:], in1=WK6[:], op=AluOp.mult)
                Q3p = pr.tile("Q3p")
                nc.vector.tensor_tensor(out=Q3p[:], in0=M7[:], in1=VK6[:], op=AluOp.mult)
                U3 = pr.tile("U3")
                nc.vector.tensor_tensor(out=U3[:], in0=EB2[:], in1=Q3[:], op=AluOp.add)
                T3 = pr.tile("T3", tag="P2")  # P2 dead after acc2a ttr
                nc.vector.tensor_tensor(out=T3[:], in0=M8[:], in1=U3[:], op=AluOp.mult)
                # side branch on gpsimd: U3' = EB2/2 + Q3', T3'' = M8*U3'
                EB2h = pr.tile("EB2h")
                nc.gpsimd.tensor_scalar(out=EB2h[:], in0=EB2[:], scalar1=0.5,
                                        scalar2=None, op0=AluOp.mult)
                U3p = pr.tile("U3p")
                nc.gpsimd.tensor_tensor(out=U3p[:], in0=EB2h[:], in1=Q3p[:], op=AluOp.add)
                T3pp = pr.tile("T3pp", tag="Q2")  # Q2 dead after acc2b ttr
                nc.vector.tensor_tensor(out=T3pp[:], in0=M8[:], in1=U3p[:], op=AluOp.mult)

            # ---- PASS 2 per pair: final projections + accs + output ----
            for pr in pairs:
                p = pr.p
                # acc3a = sum T3 * HK9 ; acc3b = sum M9 * T3''
                _proj_ttr(nc, psB, scrap, pr, "ha", 9, pr.sb["T3"], 12)
                _proj_ttr(nc, psB, scrap, pr, "dxa", 9, pr.sb["T3pp"], 16)
                # s0 = d0 @ K0
                s0 = psB.tile([128, 1], FP, tag="psB", name=f"s0_{p}")
                _mm(nc, s0[:], pr.kb, 0, pr.d0[:])
                red = pr.tile("red", dtype=FP, cols=1)
                nc.vector.tensor_reduce(
                    out=red[:], in_=pr.acc[:, 0:20], axis=mybir.AxisListType.X, op=AluOp.add
                )
                outt = pr.tile("outt", dtype=FP, cols=1)
                nc.vector.tensor_tensor(out=outt[:], in0=red[:], in1=s0[:], op=AluOp.add)
                nc.sync.dma_start(out_d[p], outt[:])

    if sanitize:
        n = _sanitize_waits(nc)
        print(f"[kernel] split {n} excess sem waits onto NOPs")
    return nc


_CACHE = {}


def _get_nc():
    if "nc" not in _CACHE:
        _CACHE["nc"] = build_nc()
    return _CACHE["nc"]


def _pack(A):
    """[B,T,U] fp32 -> [NCORES,NPAIR,128,T] fp16 (feature-major partitions)."""
    return np.ascontiguousarray(
        A.reshape(NCORES, NPAIR, 2, T, U).transpose(0, 1, 2, 4, 3)
    ).reshape(NCORES, NPAIR, 128, T).astype(np.float16)


def _marshal(X, kernel):
    Xf = np.ascontiguousarray(X, dtype=np.float32)
    tch = np.arange(T, dtype=np.float32) * (2.0 / (T - 1.0)) - 1.0
    Xa = np.empty((B, T, U), dtype=np.float32)
    Xa[:, :, 0] = tch[None, :]
    Xa[:, :, 1:] = Xf
    dXa = np.zeros_like(Xa)
    dXa[:, 1:] = Xa[:, 1:] - Xa[:, :-1]
    Ya = np.zeros_like(Xa)
    Ya[:, 1:] = Xa[:, : T - 1] - Xa[:, 0:1]
    streams = {
        "dxa": _pack(dXa),
        "za": _pack(Ya + 0.5 * dXa),
        "wa": _pack(0.5 * Ya + dXa / 6.0),
        "va": _pack(Ya / 6.0 + dXa / 24.0),
        "ha": _pack(Xa[:, T - 1 : T, :] - Xa),
    }
    d0 = (Xa[:, T - 1] - Xa[:, 0]).reshape(NCORES, NPAIR, 2 * U, 1).astype(np.float16)
    kf = np.asarray(kernel, dtype=np.float32)  # [64, 10, 64]
    kb = np.zeros((LT, 128, 128), dtype=np.float32)
    kb[:, :U, :U] = kf.transpose(1, 0, 2)
    kb[:, U:, U:] = kf.transpose(1, 0, 2)
    return streams, d0, kb.astype(np.float16)


def run(X, kernel, trace=False):
    nc = _get_nc()
    streams, d0, kb = _marshal(X, kernel)
    in_maps = []
    for c in range(NCORES):
        m = {s: streams[s][c] for s in STREAMS}
        m["d0"] = d0[c]
        m["kb"] = kb
        in_maps.append(m)
    res = run_bass_kernel_spmd(nc, in_maps, list(range(NCORES)), trace=trace)
    out = np.stack([r["out"] for r in res.results])  # [8, NPAIR, 128, 1]
    out = out.reshape(NCORES, NPAIR, 2, U).reshape(B, U)
    return out, res


def kernel(X, kernel):
    out, _ = run(X, kernel)
    return out


# revision 7
# speedup vs baseline: 1.7756x; 1.6735x over previous
"""Trainium2 Bass kernel for nn_LowRankSig_HigherOrder (v3).

Math (per example, T=2048, U=64, F=64 incl. time channel), all constants
folded into host-precomputed fp16 moving streams so the device needs only
plain tensor_tensor products (DVE 2x fp16 mode), one scan, and fused
tensor_tensor_reduce accumulations:

  dXa[t] = Xa[t]-Xa[t-1] (0 at t=0)         Ya[t] = Xa[t-1]-Xa[0] (0 at t=0)
  Za = Ya + dXa/2      (Za@Kk = E_k + M_k/2)
  Wa = Ya/2 + dXa/6    (Wa@Kk = E_k/2 + M_k/6)
  Va = Ya/6 + dXa/24   (Va@Kk = E_k/6 + M_k/24)
  Ha[t] = Xa[T-1]-Xa[t]  (Ha@Kk = reverse-exclusive-cumsum of M_k)

  out = d0@K0                                   (s0, 1-col matmul)
      + sum_t M2*(Za@K1)                        (level 1)
      + sum_t (M4*Za@K3)*(Ha@K5) + M5*(M4*Wa@K3)        (level 2, by parts)
      + sum_t T3*(Ha@K9) + M9*T3''              (level 3, by parts on EB3)
        where P3 = M7*Za@K6, EB2 = ecum(P3), Q3 = M7*Wa@K6, Q3' = M7*Va@K6,
              T3 = M8*(EB2+Q3), T3'' = M8*(EB2/2 + Q3')

Sharding: pure data parallel, 4 examples/core, 2 examples packed per 128
partitions (block-diagonal kernel). Single-pass fp16 matmuls; multi-use
projections drained PSUM->SBUF fp16 by the ACT engine; single-use
projections consumed directly from PSUM by chunked tensor_tensor_reduce.
"""

import numpy as np

import concourse.bass as bass
import concourse.mybir as mybir
import concourse.tile as tile
from concourse.bass_utils import run_bass_kernel_spmd
from bass_rust import ScopedClock


def _patched_drain_and_barrier(self, tick_clock, wait_clock):
    """Split the final drain's sem waits across multiple drain instructions
    (walrus build rejects >1-2 sync waits per instruction)."""
    drain_inst = self.nc.sync.drain()
    wait_clock.add_sem_waits(drain_inst.ins, ScopedClock({None: tick_clock.global_clock}))
    si = drain_inst.ins.sync_info
    if si is not None and si.on_wait and len(si.on_wait) > 1:
        waits = list(si.on_wait)
        ups = list(si.on_update or [])
        drain_inst.ins.sync_info = mybir.SyncInfo(on_wait=waits[:1], on_update=ups)
        for w in waits[1:]:
            d2 = self.nc.sync.drain()
            d2.ins.sync_info = mybir.SyncInfo(on_wait=[w], on_update=[])

    self.nc.all_engine_barrier()
    popped = self.nc._tile_sem_poison_stack.pop()
    assert popped is self._sem_poison
    self.nc.clear_and_free_semaphores(list(self.sems.allocated().values()))
    self.nc.all_engine_barrier()


tile.TileContext._drain_and_barrier = _patched_drain_and_barrier


def _sanitize_waits(nc, limit=1):
    """Move excess sem waits onto same-engine NOPs inserted just before."""
    import bass_rust

    counter = [0]
    for f in nc.m.functions:
        for blk in f.blocks:
            il = blk.instructions
            i = 0
            while i < len(il):
                inst = il[i]
                si = inst.sync_info
                waits = list(si.on_wait) if (si is not None and si.on_wait) else []
                if len(waits) > limit:
                    keep, extra = waits[:limit], waits[limit:]
                    inst.sync_info = mybir.SyncInfo(
                        on_wait=keep, on_update=list(si.on_update or [])
                    )
                    for j in range(0, len(extra), limit):
                        counter[0] += 1
                        nop = bass_rust.InstNoOp(
                            name=f"waitnop-{counter[0]}", ins=[], outs=[]
                        )
                        nop.engine = inst.engine
                        nop.sync_info = mybir.SyncInfo(
                            on_wait=extra[j : j + limit], on_update=[]
                        )
                        il.insert(i, nop)
                        i += 1
                i += 1
    return counter[0]


B, T, FX = 32, 2048, 63
U = 64
LT = 10
NCORES = 8
BL = B // NCORES  # 4 examples per core
NPAIR = BL // 2   # 2 partition-packed pairs per core

FP = mybir.dt.float32
F16 = mybir.dt.float16
AluOp = mybir.AluOpType

STREAMS = ["dxa", "za", "wa", "va", "ha"]


def _mm(nc, ps_ap, kb, k, rhs_ap):
    nc.tensor.matmul(ps_ap, kb[:, k * 128 : (k + 1) * 128], rhs_ap, start=True, stop=True)


class Pair:
    """Per-pair tile state."""

    def __init__(self, nc, pool, kb, dr, p):
        self.nc, self.pool, self.kb, self.p = nc, pool, kb, p
        # stream tiles, DMA'd in halves
        self.st = {}
        for s in STREAMS:
            t = pool.tile([128, T], F16, tag=f"{s}{p}", name=f"{s}{p}")
            for h in range(2):
                sl = slice(h * 1024, (h + 1) * 1024)
                nc.sync.dma_start(t[:, sl], dr[s][p][:, sl])
            self.st[s] = t
        self.d0 = pool.tile([128, 1], F16, tag=f"d0_{p}", name=f"d0_{p}")
        nc.sync.dma_start(self.d0[:], dr["d0"][p])
        self.acc = pool.tile([128, 24], FP, tag=f"acc{p}", name=f"acc{p}")
        self.sb = {}

    def tile(self, nm, dtype=F16, cols=T, bufs=None, tag=None):
        t = self.pool.tile([128, cols], dtype, tag=f"{tag or nm}{self.p}",
                           name=f"{nm}{self.p}", bufs=bufs)
        self.sb[nm] = t
        return t


def _proj_copied(nc, psA, pair, nm, stream, k):
    """Project stream@Kk, drain PSUM -> SBUF fp16 via ACT. Returns SBUF tile."""
    dst = pair.tile(nm)
    src = pair.st[stream]
    for h in range(2):
        ps = psA.tile([128, 1024], FP, tag="psA", name=f"psA_{nm}{pair.p}")
        for j in range(2):
            lo = h * 1024 + j * 512
            _mm(nc, ps[:, j * 512 : (j + 1) * 512], pair.kb, k, src[:, lo : lo + 512])
        nc.scalar.copy(out=dst[:, h * 1024 : (h + 1) * 1024], in_=ps[:])
    return dst


def _proj_ttr(nc, psB, scrap, pair, stream, k, other_sb, cols):
    """Project stream@Kk into PSUM chunks and immediately reduce
    sum_t(proj * other_sb) into acc[:, cols] via tensor_tensor_reduce."""
    src = pair.st[stream]
    for j in range(4):
        lo = j * 512
        ps = psB.tile([128, 512], FP, tag="psB", name=f"psB_{stream}{k}_{pair.p}")
        _mm(nc, ps[:], pair.kb, k, src[:, lo : lo + 512])
        sc = scrap.tile([128, 512], F16, tag="scrap", name=f"sc_{stream}{k}_{j}_{pair.p}")
        nc.vector.scalar_tensor_tensor(
            out=sc[:], in0=ps[:], scalar=1.0, in1=other_sb[:, lo : lo + 512],
            op0=AluOp.mult, op1=AluOp.mult,
            accum_out=pair.acc[:, cols + j : cols + j + 1],
        )


def build_nc(sanitize=True):
    nc = bass.Bass("TRN2", target_bir_lowering=False, debug=False)
    dr = {}
    for s in STREAMS:
        dr[s] = nc.dram_tensor(s, [NPAIR, 128, T], F16, kind="ExternalInput")
    dr["d0"] = nc.dram_tensor("d0", [NPAIR, 128, 1], F16, kind="ExternalInput")
    kb_d = nc.dram_tensor("kb", [LT, 128, 128], F16, kind="ExternalInput")
    out_d = nc.dram_tensor("out", [NPAIR, 128, 1], FP, kind="ExternalOutput")

    with tile.TileContext(nc) as tc:
        with (
            tc.tile_pool(name="pool", bufs=1) as pool,
            tc.tile_pool(name="scrap", bufs=4) as scrap,
            tc.tile_pool(name="psA", bufs=3, space="PSUM") as psA,
            tc.tile_pool(name="psB", bufs=2, space="PSUM") as psB,
        ):
            kb = pool.tile([128, LT * 128], F16, tag="kb", name="kb")
            nc.sync.dma_start(
                kb[:].rearrange("p (k m) -> p k m", k=LT),
                kb_d.ap().rearrange("k f m -> f k m"),
            )
            zeros = pool.tile([128, T], F16, tag="zeros", name="zeros")
            nc.gpsimd.memset(zeros[:], 0.0)

            pairs = [Pair(nc, pool, kb, dr, p) for p in range(NPAIR)]

            # ---- PASS 1 per pair ----
            for pr in pairs:
                p = pr.p
                # copied projections, phase 1
                M4 = _proj_copied(nc, psA, pr, "M4", "dxa", 4)
                ZK3 = _proj_copied(nc, psA, pr, "ZK3", "za", 3)
                WK3 = _proj_copied(nc, psA, pr, "WK3", "wa", 3)
                ZK1 = _proj_copied(nc, psA, pr, "ZK1", "za", 1)
                # level 1: acc1 = sum M2 * ZK1   (M2 direct from PSUM)
                _proj_ttr(nc, psB, scrap, pr, "dxa", 2, ZK1, 0)
                # level 2 products
                P2 = pr.tile("P2")
                nc.vector.tensor_tensor(out=P2[:], in0=M4[:], in1=ZK3[:], op=AluOp.mult)
                Q2 = pr.tile("Q2")
                nc.vector.tensor_tensor(out=Q2[:], in0=M4[:], in1=WK3[:], op=AluOp.mult)
                # acc2b = sum M5 * Q2 ; acc2a = sum HK5 * P2  (M5, HK5 direct)
                _proj_ttr(nc, psB, scrap, pr, "dxa", 5, Q2, 4)
                _proj_ttr(nc, psB, scrap, pr, "ha", 5, P2, 8)
                # copied projections, phase 2
                M7 = _proj_copied(nc, psA, pr, "M7", "dxa", 7)
                ZK6 = _proj_copied(nc, psA, pr, "ZK6", "za", 6)
                WK6 = _proj_copied(nc, psA, pr, "WK6", "wa", 6)
                VK6 = _proj_copied(nc, psA, pr, "VK6", "va", 6)
                M8 = _proj_copied(nc, psA, pr, "M8", "dxa", 8)
                # level 3 chain
                P3 = pr.tile("P3")
                nc.vector.tensor_tensor(out=P3[:], in0=M7[:], in1=ZK6[:], op=AluOp.mult)
                EB2 = pr.tile("EB2")
                nc.gpsimd.memset(EB2[:, 0:1], 0.0)
                nc.vector.tensor_tensor_scan(
                    out=EB2[:, 1:T], data0=P3[:, 0 : T - 1], data1=zeros[:, 0 : T - 1],
                    initial=0.0, op0=AluOp.add, op1=AluOp.add,
                )
                Q3 = pr.tile("Q3")
                nc.vector.tensor_tensor(out=Q3[:], in0=M7[:], in1=WK6[:], op=AluOp.mult)
                Q3p = pr.tile("Q3p")
                nc.vector.tensor_tensor(out=Q3p[:], in0=M7[:], in1=VK6[:], op=AluOp.mult)
                U3 = pr.tile("U3")
                nc.vector.tensor_tensor(out=U3[:], in0=EB2[:], in1=Q3[:], op=AluOp.add)
                T3 = pr.tile("T3", tag="P2")  # P2 dead after acc2a ttr
                nc.vector.tensor_tensor(out=T3[:], in0=M8[:], in1=U3[:], op=AluOp.mult)
                # side branch: U3' = EB2/2 + Q3' (one 1x stt on DVE)
                U3p = pr.tile("U3p")
                nc.vector.scalar_tensor_tensor(
                    out=U3p[:], in0=EB2[:], scalar=0.5, in1=Q3p[:],
                    op0=AluOp.mult, op1=AluOp.add,
                )
                T3pp = pr.tile("T3pp", tag="Q2")  # Q2 dead after acc2b ttr
                nc.vector.tensor_tensor(out=T3pp[:], in0=M8[:], in1=U3p[:], op=AluOp.mult)

            # ---- PASS 2 per pair: final projections + accs + output ----
            for pr in pairs:
                p = pr.p
                # acc3a = sum T3 * HK9 ; acc3b = sum M9 * T3''
                _proj_ttr(nc, psB, scrap, pr, "ha", 9, pr.sb["T3"], 12)
                _proj_ttr(nc, psB, scrap, pr, "dxa", 9, pr.sb["T3pp"], 16)
                # s0 = d0 @ K0
                s0 = psB.tile([128, 1], FP, tag="psB", name=f"s0_{p}")
                _mm(nc, s0[:], pr.kb, 0, pr.d0[:])
                red = pr.tile("red", dtype=FP, cols=1)
                nc.vector.tensor_reduce(
                    out=red[:], in_=pr.acc[:, 0:20], axis=mybir.AxisListType.X, op=AluOp.add
                )
                outt = pr.tile("outt", dtype=FP, cols=1)
                nc.vector.tensor_tensor(out=outt[:], in0=red[:], in1=s0[:], op=AluOp.add)
                nc.sync.dma_start(out_d[p], outt[:])

    if sanitize:
        n = _sanitize_waits(nc)
        print(f"[kernel] split {n} excess sem waits onto NOPs")
    return nc


_CACHE = {}


def _get_nc():
    if "nc" not in _CACHE:
        _CACHE["nc"] = build_nc()
    return _CACHE["nc"]


def _pack(A):
    """[B,T,U] fp32 -> [NCORES,NPAIR,128,T] fp16 (feature-major partitions)."""
    return np.ascontiguousarray(
        A.reshape(NCORES, NPAIR, 2, T, U).transpose(0, 1, 2, 4, 3)
    ).reshape(NCORES, NPAIR, 128, T).astype(np.float16)


def _marshal(X, kernel):
    Xf = np.ascontiguousarray(X, dtype=np.float32)
    tch = np.arange(T, dtype=np.float32) * (2.0 / (T - 1.0)) - 1.0
    Xa = np.empty((B, T, U), dtype=np.float32)
    Xa[:, :, 0] = tch[None, :]
    Xa[:, :, 1:] = Xf
    dXa = np.zeros_like(Xa)
    dXa[:, 1:] = Xa[:, 1:] - Xa[:, :-1]
    Ya = np.zeros_like(Xa)
    Ya[:, 1:] = Xa[:, : T - 1] - Xa[:, 0:1]
    streams = {
        "dxa": _pack(dXa),
        "za": _pack(Ya + 0.5 * dXa),
        "wa": _pack(0.5 * Ya + dXa / 6.0),
        "va": _pack(Ya / 6.0 + dXa / 24.0),
        "ha": _pack(Xa[:, T - 1 : T, :] - Xa),
    }
    d0 = (Xa[:, T - 1] - Xa[:, 0]).reshape(NCORES, NPAIR, 2 * U, 1).astype(np.float16)
    kf = np.asarray(kernel, dtype=np.float32)  # [64, 10, 64]
    kb = np.zeros((LT, 128, 128), dtype=np.float32)
    kb[:, :U, :U] = kf.transpose(1, 0, 2)
    kb[:, U:, U:] = kf.transpose(1, 0, 2)
    return streams, d0, kb.astype(np.float16)


def run(X, kernel, trace=False):
    nc = _get_nc()
    streams, d0, kb = _marshal(X, kernel)
    in_maps = []
    for c in range(NCORES):
        m = {s: streams[s][c] for s in STREAMS}
        m["d0"] = d0[c]
        m["kb"] = kb
        in_maps.append(m)
    res = run_bass_kernel_spmd(nc, in_maps, list(range(NCORES)), trace=trace)
    out = np.stack([r["out"] for r in res.results])  # [8, NPAIR, 128, 1]
    out = out.reshape(NCORES, NPAIR, 2, U).reshape(B, U)
    return out, res


def kernel(X, kernel):
    out, _ = run(X, kernel)
    return out
